# revision 20
# baseline (speedup 1.0000x reference)
"""Trainium2 Bass kernel for the Dempster-Shafer sequential-combination layer.

Math (per batch element; inputs m[p, k], p=0..63 prototypes, k=0..10 with
slot 10 = omega):
    The reference left-fold is linear in the running state and per-step
    normalization is a uniform positive scale, so intermediate
    normalizations cancel.  With y = M / M_w (state scaled by the running
    omega product) the fold is y' = (Q + 1/3) y + Q, Q = m/(3w).  Shifting
    to z = y + 1 gives
        z' = (Q + 1/3) z + 2/3
    i.e. a tensor_tensor_scan whose d1 operand is a CONSTANT except at the
    64-element chain starts, where d0 = 0 / d1 = 3*(Q0 + 1/3) = m0/w0 + 1
    resets the recurrence.  Final: out_k = (z_k - 1) * r, out_omega = r,
    r = 1 / (sum_k z_k - 9).

Engine choreography per chunk (8 batch groups x 64 prototypes x 11 slots
per partition; 32 chunks per core; DMA floor ~8.3 us/chunk):
  - Scalar: u = exp(-ln(3w+delta)) (two LUT passes, activation table
    pinned once via a manual LoadActFuncSet so no per-chunk table loads),
    then d0 = Q + 1/3 as a TRANSPOSING activation copy (reads Q in m-layout,
    writes chain layout) — the transpose's strided write lives on the
    Scalar engine, so the DVE multiply writes contiguously.
  - DVE: Q = m * u with contiguous writes; a slice of the scan; the tiny
    reduce/reciprocal epilogue head.
  - GpSimd: d1 chain-start fixups, d0 chain-start zeroing, the rest of
    the scan, and the output assembly.
  - d1 is a persistent constant tile (2/3) with per-chunk chain-start
    writes; two parity copies avoid serializing consecutive chunks.
"""

import numpy as np

B = 262144
P = 64
K = 11
KC = K - 1             # chains per group (omega chain == 1 identically)
N_CORES = 8
B_CORE = B // N_CORES  # 32768
NB = 4                 # batch groups per partition per chunk
DELTA = 1e-12          # guards w == 0
THIRD = float(np.float32(1.0) / np.float32(3.0))
TWO_THIRD = float(np.float32(2.0) / np.float32(3.0))
YS = 12                # chains (of NB*KC=40) scanned on DVE; rest on GpSimd
BUFS = 4
ACT_SET_LN_EXP = 6     # act_func_sets index of natural_log_exp_and_others

_CACHE = {}


def _build_program(reps=1, nb=NB, bufs=BUFS, recip_mode="explog",
                   epi="gpsimd", w1q=120, compute="full",
                   outdma="skewscalar", indma="sync", skew=1, out_skew=3):
    import concourse.bacc as bacc
    import concourse.mybir as mybir
    from concourse.tile import TileContext

    f32 = mybir.dt.float32
    Alu = mybir.AluOpType
    Act = mybir.ActivationFunctionType

    nchains = nb * KC
    n_chunks = B_CORE // (128 * nb)
    nc = bacc.Bacc(
        "TRN2", target_bir_lowering=False, debug=False, num_devices=N_CORES
    )
    x = nc.declare_dram_parameter("x", [B_CORE, P * K], f32, isOutput=False)
    out = nc.declare_dram_parameter("out", [B_CORE, K], f32, isOutput=True)

    xv = x.rearrange("(c i g) d -> c i (g d)", i=128, g=nb)
    ov = out.rearrange("(c i g) d -> c i (g d)", i=128, g=nb)

    with TileContext(nc) as tc:
        with tc.tile_pool(name="consts", bufs=1) as cpool, \
             tc.tile_pool(name="p", bufs=bufs) as pool:
            delta_ = cpool.tile([128, 1], f32, name="delta_")
            nc.gpsimd.memset(delta_[:], DELTA)
            if recip_mode == "explog":
                nc.scalar.add_instruction(mybir.InstLoadActFuncSet(
                    name=nc.get_next_instruction_name(),
                    act_func_set_id=ACT_SET_LN_EXP, ins=[], outs=[]))
            # persistent d1 tiles (parity pair): 2/3 everywhere; chain-start
            # slots rewritten per chunk
            d1s = []
            for pi in range(2):
                d1_ = cpool.tile([128, nb * KC * P], f32, name=f"d1{pi}_")
                nc.gpsimd.memset(d1_[:], TWO_THIRD)
                d1s.append(d1_)

            pending_out = []   # (dram_view, sbuf_tile) awaiting skewed out-DMA
            state = {}         # chunk idx -> tiles handed from front to back

            def front(i):
                """DMA-in, reciprocal, and the Q multiply for chunk i."""
                c = i % n_chunks
                m_ = pool.tile([128, nb * P * K], f32, name="m_")
                u_ = pool.tile([128, nb * P], f32, name="u_")
                u2_ = pool.tile([128, nb * P], f32, name="u2_")
                q_ = pool.tile([128, nb * P * KC], f32, name="q_")
                d0_ = pool.tile([128, nb * KC * P], f32, name="d0_")
                s_ = pool.tile([128, nb], f32, name="s_")
                r_ = pool.tile([128, nb], f32, name="r_")
                o_ = pool.tile([128, nb * K], f32, name="o_")

                if outdma == "skewscalar" and len(pending_out) >= out_skew:
                    # an old chunk's result is long since ready: issuing it
                    # on the scalar HWDGE queue costs no engine time
                    dst, src = pending_out.pop(0)
                    nc.scalar.dma_start(out=dst, in_=src[:])

                nc.sync.dma_start(out=m_[:], in_=xv[c])

                m4 = m_.rearrange("p (g q k) -> p g q k", g=nb, q=P, k=K)
                # u2 = 1/(3w + delta)
                if recip_mode == "explog":
                    nc.scalar.activation(
                        out=u_[:], in_=m4[:, :, :, K - 1],
                        func=Act.Ln, bias=delta_[:], scale=3.0,
                    )
                    nc.scalar.activation(
                        out=u2_[:], in_=u_[:],
                        func=Act.Exp, bias=0.0, scale=-1.0,
                    )
                else:
                    nc.scalar.activation(
                        out=u_[:], in_=m4[:, :, :, K - 1],
                        func=Act.Copy, bias=DELTA, scale=3.0,
                    )
                    nc.vector.reciprocal_approx_fast(out=u2_[:], in_=u_[:])

                # Q[g, q, k] = m[g, q, k] * u[g, q]: contiguous write
                q4 = q_.rearrange("p (g q k) -> p g q k", g=nb, q=P, k=KC)
                mw = m4[:, :, :, :KC]
                uw = (
                    u2_.rearrange("p (g q) -> p g q", g=nb)
                    .unsqueeze(3)
                    .broadcast_to([128, nb, P, KC])
                )
                # split at q-row granularity: first w1q rows on DVE, rest
                # on GpSimd (a row = KC elems)
                wq = max(0, min(nb * P, w1q))
                g0, qr = divmod(wq, P)
                if g0 > 0:
                    nc.vector.tensor_tensor(
                        out=q4[:, :g0], in0=mw[:, :g0],
                        in1=uw[:, :g0], op=Alu.mult,
                    )
                if qr > 0:
                    nc.vector.tensor_tensor(
                        out=q4[:, g0, :qr], in0=mw[:, g0, :qr],
                        in1=uw[:, g0, :qr], op=Alu.mult,
                    )
                if wq < nb * P:
                    if qr > 0:
                        nc.gpsimd.tensor_tensor(
                            out=q4[:, g0, qr:], in0=mw[:, g0, qr:],
                            in1=uw[:, g0, qr:], op=Alu.mult,
                        )
                    if g0 + 1 < nb or (qr == 0 and g0 < nb):
                        ghi = g0 + (1 if qr > 0 else 0)
                        nc.gpsimd.tensor_tensor(
                            out=q4[:, ghi:], in0=mw[:, ghi:],
                            in1=uw[:, ghi:], op=Alu.mult,
                        )
                state[i] = (q_, q4, d0_, s_, r_, o_)

            def back(i):
                """d0 build, scan, and epilogue for chunk i."""
                c = i % n_chunks
                q_, q4, d0_, s_, r_, o_ = state.pop(i)
                d1_ = d1s[i % 2]

                # d0[g, k, q] = Q[g, q, k] + 1/3 — transposing write on
                # the Scalar engine
                d04 = d0_.rearrange("p (g k q) -> p g k q", g=nb, k=KC, q=P)
                d0t = d04.transpose([0, 1, 3, 2])    # [128, nb, P, KC]
                nc.scalar.activation(
                    out=d0t, in_=q4, func=Act.Copy, bias=THIRD, scale=1.0,
                )

                # chain starts: d1 = 3*d0 (= m0/w0 + 1), then d0 = 0.
                # (Pool rejects TensorScalarPtr encodings on HW, so the
                # scale-by-3 runs as a scalar-engine copy.)
                d14 = d1_.rearrange("p (g k q) -> p g k q", g=nb, k=KC, q=P)
                nc.scalar.activation(
                    out=d14[:, :, :, 0], in_=d04[:, :, :, 0],
                    func=Act.Copy, bias=0.0, scale=3.0,
                )
                nc.gpsimd.memset(d04[:, :, :, 0:1], 0.0)

                # z[t] = d0[t] * z[t-1] + d1[t] — DVE only (the scan opcode
                # is not in the Pool engine's ISA); out reuses q_ (dead)
                y_ = q_
                nc.vector.tensor_tensor_scan(
                    out=y_[:], data0=d0_[:], data1=d1_[:],
                    initial=0.0, op0=Alu.mult, op1=Alu.add,
                )

                y4 = y_.rearrange("p (g k q) -> p g k q", g=nb, k=KC, q=P)
                fin = y4[:, :, :, P - 1]  # [128, nb, KC] = z finals
                nc.vector.tensor_reduce(
                    out=s_[:], in_=fin, axis=mybir.AxisListType.X,
                    op=Alu.add,
                )
                # r = 1 / (S - 9)  (= 1 / (1 + sum_k y_k))
                nc.vector.tensor_scalar(
                    out=s_[:], in0=s_[:], scalar1=-9.0, scalar2=None,
                    op0=Alu.add,
                )
                nc.vector.reciprocal_approx_fast(out=r_[:], in_=s_[:])

                # out_k = (z_k - 1) * r;  out_omega = r
                rb = (
                    r_.rearrange("p g -> p g")
                    .unsqueeze(2)
                    .broadcast_to([128, nb, KC])
                )
                o3 = o_.rearrange("p (g k) -> p g k", g=nb)
                nc.vector.scalar_tensor_tensor(
                    out=o3[:, :, :KC], in0=fin, scalar=1.0, in1=rb,
                    op0=Alu.subtract, op1=Alu.mult,
                )
                nc.vector.tensor_copy(
                    out=o3[:, :, KC:], in_=r_[:].unsqueeze(2)
                )

                if outdma == "skewscalar":
                    pending_out.append((ov[c], o_))
                else:
                    out_eng = {"sync": nc.sync,
                               "gpsimd": nc.gpsimd,
                               "scalar": nc.scalar}[outdma]
                    out_eng.dma_start(out=ov[c], in_=o_[:])

            total = reps * n_chunks
            if compute == "dmaonly":
                for i in range(total):
                    c = i % n_chunks
                    m_ = pool.tile([128, nb * P * K], f32, name="m_")
                    o_ = pool.tile([128, nb * K], f32, name="o_")
                    nc.sync.dma_start(out=m_[:], in_=xv[c])
                    nc.vector.tensor_copy(out=o_[:], in_=m_[:, : nb * K])
                    nc.sync.dma_start(out=ov[c], in_=o_[:])
            else:
                for i in range(total + skew):
                    if i < total:
                        front(i)
                    if i >= skew:
                        back(i - skew)
                for dst, src in pending_out:
                    nc.scalar.dma_start(out=dst, in_=src[:])

    nc.compile()
    return nc


def kernel(inputs: np.ndarray) -> np.ndarray:
    from concourse.bass_utils import run_bass_kernel_spmd

    if "nc" not in _CACHE:
        _CACHE["nc"] = _build_program()
    nc = _CACHE["nc"]

    x = np.ascontiguousarray(np.asarray(inputs, dtype=np.float32)).reshape(
        B, P * K
    )
    shards = x.reshape(N_CORES, B_CORE, P * K)
    in_maps = [{"x": shards[i]} for i in range(N_CORES)]
    res = run_bass_kernel_spmd(nc, in_maps, core_ids=list(range(N_CORES)))
    outs = [res.results[i]["out"] for i in range(N_CORES)]
    return np.concatenate(outs, axis=0).reshape(B, K)
